# revision 8
# baseline (speedup 1.0000x reference)
"""Trainium2 Bass kernel for nn_ComparisonLayer.

Computes, for x:(L,B,D) with L=512,B=2,D=256,C=128,O=64:
    xb  = layernorm(transpose(x,(1,0,2)))          # (B,L,D)
    a   = xb@w1+b1 ; b = xb@w2+b2                  # (B,L,C)
    out[b,l,m,o] = sum_c a[b,l,c]*b[b,m,c]*w3[c,o] + b3[o]
                   + (a@w4)[b,l,o] - (b@w4)[b,m,o] # (B,L,L,O)

Sharding: 8 cores, core k handles batch k//4 and l-block (k%4)*128.
Each core writes out[b, lblk:lblk+128, :, :] = (128, 512*64) with l on
partitions and (m,o) contiguous on the free dim -> contiguous HBM writes.

Per chunk (8 m's = 512 free columns) three matmuls accumulate in PSUM:
  1. main : lhsT=aT (C=128,l=128), rhs=R_chunk[c,(m,o)] = bT[c,m]*w3[c,o]
  2. a4   : lhsT=a4T (O=64,l=128), rhs=I64 tiled (constant)  -> +a4[l,o]
  3. row  : lhsT=ones (1,128),     rhs=(b3-b4) flat slice    -> +b3[o]-b4[m,o]
R_chunk is built on the vector engine with stride-0 broadcast APs.
"""

import os
import numpy as np
import ml_dtypes

import concourse.bacc as bacc
import concourse.bass as bass
import concourse.mybir as mybir
import concourse.tile as tile
from concourse.bass_utils import run_bass_kernel_spmd

L, B, D, C, O = 512, 2, 256, 128, 64
NCORES = 8
LBLK = 128                   # l rows per core
MOCT = 8                     # m values per chunk
CHW = MOCT * O               # 512 = chunk width (free columns)
NCH = L // MOCT              # 64 chunks
LN_EPS = 1e-5

F32 = mybir.dt.float32
BF16 = mybir.dt.bfloat16
F32R = mybir.dt.float32r

# "f32r" (default): matmul operands float32r (full PE rate at N>=512,
# near-fp32 accuracy). "bf16": operands bf16.
MM_MODE = os.environ.get("BASS_MM_MODE", "f32r")


def _build(mode: str):
    # storage dtype of main-matmul operands; producers round on write
    cdt = F32R if mode == "f32r" else BF16
    npdt = np.float32 if mode == "f32r" else ml_dtypes.bfloat16

    nc = bacc.Bacc("TRN2", target_bir_lowering=False, debug=False)

    xb_d = nc.dram_tensor("xb", (L, D), F32, kind="ExternalInput")
    xa_d = nc.dram_tensor("xa", (LBLK, D), F32, kind="ExternalInput")
    w1_d = nc.dram_tensor("w1g", (D, C), F32, kind="ExternalInput")
    w2_d = nc.dram_tensor("w2g", (D, C), F32, kind="ExternalInput")
    b1_d = nc.dram_tensor("b1e", (C, 1), F32, kind="ExternalInput")
    b2_d = nc.dram_tensor("b2e", (C, 1), F32, kind="ExternalInput")
    w3_d = nc.dram_tensor("w3c", (C, O), F32, kind="ExternalInput")
    w4_d = nc.dram_tensor("w4f", (C, O), F32, kind="ExternalInput")
    b3_d = nc.dram_tensor("b3f", (1, O), F32, kind="ExternalInput")
    id128_d = nc.dram_tensor("id128", (128, 128), F32, kind="ExternalInput")
    i64r_d = nc.dram_tensor("i64rep", (O, CHW), cdt, kind="ExternalInput")
    out_d = nc.dram_tensor("out", (LBLK, L * O), F32, kind="ExternalOutput")

    NT = L // 128  # xb tiles

    with tile.TileContext(nc) as tc:
        with (
            tc.tile_pool(name="const", bufs=1) as cp,
            tc.tile_pool(name="work", bufs=2) as wp,
            tc.tile_pool(name="rpool", bufs=6) as rp,
            tc.tile_pool(name="opool", bufs=6) as op,
            tc.tile_pool(name="ps_pre", bufs=4, space="PSUM") as pp,
            tc.tile_pool(name="ps_main", bufs=4, space="PSUM") as pm,
        ):
            # ---------- loads ----------
            xsb = cp.tile([128, NT, D], F32)
            nc.sync.dma_start(xsb[:], xb_d.rearrange("(t p) d -> p t d", p=128))
            xasb = cp.tile([128, D], F32)
            nc.sync.dma_start(xasb[:], xa_d[:])
            w1s = cp.tile([128, 2, C], F32)
            nc.sync.dma_start(w1s[:], w1_d.rearrange("(h p) c -> p h c", p=128))
            w2s = cp.tile([128, 2, C], F32)
            nc.sync.dma_start(w2s[:], w2_d.rearrange("(h p) c -> p h c", p=128))
            b1s = cp.tile([C, 1], F32)
            nc.sync.dma_start(b1s[:], b1_d[:])
            b2s = cp.tile([C, 1], F32)
            nc.sync.dma_start(b2s[:], b2_d[:])
            w3s = cp.tile([C, O], F32)
            nc.sync.dma_start(w3s[:], w3_d[:])
            w4s = cp.tile([C, O], F32)
            nc.sync.dma_start(w4s[:], w4_d[:])
            b3s = cp.tile([1, O], F32)
            nc.sync.dma_start(b3s[:], b3_d[:])
            id128 = cp.tile([128, 128], F32)
            nc.sync.dma_start(id128[:], id128_d[:])
            i64r = cp.tile([O, CHW], cdt)
            nc.sync.dma_start(i64r[:], i64r_d[:])

            epsp = cp.tile([128, 1], F32)
            nc.vector.memset(epsp[:], LN_EPS)
            zerop = cp.tile([128, 1], F32)
            nc.vector.memset(zerop[:], 0.0)
            ones_f = cp.tile([1, 128], F32)
            nc.vector.memset(ones_f[:], 1.0)
            ones_c = cp.tile([1, 128], cdt)
            nc.vector.tensor_copy(ones_c[:], ones_f[:])

            # ---------- layernorm ----------
            def layer_norm(dst, src):
                # dst, src: (128, D) f32
                s = wp.tile([128, 1], F32, tag="ln_s")
                nc.vector.tensor_reduce(
                    s[:], src, axis=mybir.AxisListType.X, op=mybir.AluOpType.add
                )
                nmu = wp.tile([128, 1], F32, tag="ln_nmu")
                nc.scalar.mul(nmu[:], s[:], -1.0 / D)
                cen = wp.tile([128, D], F32, tag="ln_cen")
                nc.scalar.activation(
                    cen[:], src, mybir.ActivationFunctionType.Identity,
                    bias=nmu[:],
                )
                sq = wp.tile([128, D], F32, tag="ln_sq")
                vs = wp.tile([128, 1], F32, tag="ln_vs")
                nc.scalar.activation(
                    sq[:], cen[:], mybir.ActivationFunctionType.Square,
                    bias=zerop[:], accum_out=vs[:],
                )
                std = wp.tile([128, 1], F32, tag="ln_std")
                nc.scalar.activation(
                    std[:], vs[:], mybir.ActivationFunctionType.Sqrt,
                    bias=epsp[:], scale=1.0 / D,
                )
                rstd = wp.tile([128, 1], F32, tag="ln_rstd")
                nc.vector.reciprocal(rstd[:], std[:])
                nc.scalar.activation(
                    dst, cen[:], mybir.ActivationFunctionType.Copy,
                    scale=rstd[:],
                )

            xn = cp.tile([128, NT, D], F32)
            for t in range(NT):
                layer_norm(xn[:, t, :], xsb[:, t, :])
            xna = cp.tile([128, D], F32)
            layer_norm(xna[:], xasb[:])

            # ---------- transposes: xnT (d on partitions) ----------
            xnT = cp.tile([128, 2, L], F32)     # [d_in_half, h, m]
            for t in range(NT):
                for h in range(2):
                    tp = pp.tile([128, 128], F32, tag="pre")
                    nc.tensor.transpose(
                        tp[:], xn[:, t, h * 128:(h + 1) * 128], id128[:]
                    )
                    nc.scalar.copy(xnT[:, h, t * 128:(t + 1) * 128], tp[:])
            xnaT = cp.tile([128, 2, 128], F32)
            for h in range(2):
                tp = pp.tile([128, 128], F32, tag="pre")
                nc.tensor.transpose(
                    tp[:], xna[:, h * 128:(h + 1) * 128], id128[:]
                )
                nc.scalar.copy(xnaT[:, h, :], tp[:])

            # ---------- bT (C, L) and aT (C, 128), fp32 ----------
            bps = pp.tile([C, L], F32, tag="pre")
            for h in range(2):
                nc.tensor.matmul(
                    bps[:], w2s[:, h, :], xnT[:, h, :],
                    start=(h == 0), stop=(h == 1),
                )
            bT = cp.tile([C, L], F32)
            nc.vector.tensor_scalar_add(bT[:], bps[:], b2s[:])

            aps_ = pp.tile([C, 128], F32, tag="pre")
            for h in range(2):
                nc.tensor.matmul(
                    aps_[:], w1s[:, h, :], xnaT[:, h, :],
                    start=(h == 0), stop=(h == 1),
                )
            aT = cp.tile([C, 128], F32)
            nc.vector.tensor_scalar_add(aT[:], aps_[:], b1s[:])

            # main-matmul lhsT in mm dtype (rounded on write)
            aT_c = cp.tile([C, 128], cdt)
            nc.vector.tensor_copy(aT_c[:], aT[:])
            bT_c = bT   # only ever a DVE tensor_tensor input

            # ---------- a4 (l,o) -> a4T (o,l) ----------
            a4ps = pp.tile([128, O], F32, tag="pre")
            nc.tensor.matmul(a4ps[:], aT[:], w4s[:], start=True, stop=True)
            a4sb = cp.tile([128, O], F32)
            nc.scalar.copy(a4sb[:], a4ps[:])
            a4Tps = pp.tile([O, 128], F32, tag="pre")
            nc.tensor.transpose(a4Tps[:], a4sb[:], id128[:])
            a4T = cp.tile([O, 128], cdt)
            nc.scalar.copy(a4T[:], a4Tps[:])

            # ---------- row64 = (b3 - b4) flattened (1, L*O) ----------
            b3cps = pp.tile([128, O], F32, tag="pre")
            nc.tensor.matmul(b3cps[:], ones_f[:], b3s[:], start=True, stop=True)
            b3c = cp.tile([128, O], F32)
            nc.scalar.copy(b3c[:], b3cps[:])

            negb4 = cp.tile([128, NT, O], cdt)
            for mt in range(NT):
                b4ps = pp.tile([128, O], F32, tag="pre")
                nc.tensor.matmul(
                    b4ps[:], bT[:, mt * 128:(mt + 1) * 128], w4s[:],
                    start=True, stop=True,
                )
                nc.vector.tensor_sub(negb4[:, mt, :], b3c[:], b4ps[:])

            row64 = cp.tile([1, L * O], cdt)
            for mt in range(NT):
                dst = row64[0:1, mt * 128 * O:(mt + 1) * 128 * O]
                nc.gpsimd.dma_start(dst, negb4[:, mt, :])

            # ---------- main loop over chunks ----------
            for ch in range(NCH):
                rch = rp.tile([C, MOCT, O], cdt, tag="rch")
                in0 = bT_c[:, ch * MOCT:(ch + 1) * MOCT].unsqueeze(2) \
                    .broadcast_to((C, MOCT, O))
                in1 = w3s[:].unsqueeze(1).broadcast_to((C, MOCT, O))
                nc.vector.tensor_mul(rch[:], in0, in1)

                ps = pm.tile([128, CHW], F32, tag="ps")
                nc.tensor.matmul(ps[:], aT_c[:], rch[:],
                                 start=True, stop=False)
                nc.tensor.matmul(ps[:], a4T[:], i64r[:],
                                 start=False, stop=False)
                nc.tensor.matmul(
                    ps[:], ones_c[:],
                    row64[0:1, ch * CHW:(ch + 1) * CHW],
                    start=False, stop=True,
                )

                ob = op.tile([128, CHW], F32, tag="ob")
                nc.scalar.copy(ob[:], ps[:])
                nc.sync.dma_start(out_d[:, ch * CHW:(ch + 1) * CHW], ob[:])

    nc.compile()
    return nc, npdt


_CACHE = {}


def _get_nc(mode):
    if mode not in _CACHE:
        _CACHE[mode] = _build(mode)
    return _CACHE[mode]


def _make_in_maps(x, ln_gamma, ln_beta, w1, b1, w2, b2, w3, b3, w4, npdt):
    x = np.ascontiguousarray(x, dtype=np.float32)
    g = np.asarray(ln_gamma, np.float32)
    be = np.asarray(ln_beta, np.float32)
    w1 = np.asarray(w1, np.float32)
    w2 = np.asarray(w2, np.float32)
    # fold the LN affine into the first-layer weights:
    # (xn*g + be) @ w = xn @ (g[:,None]*w) + be @ w
    w1g = g[:, None] * w1
    w2g = g[:, None] * w2
    b1e = (np.asarray(b1, np.float32) + be @ w1).reshape(C, 1)
    b2e = (np.asarray(b2, np.float32) + be @ w2).reshape(C, 1)
    w3c = np.ascontiguousarray(np.asarray(w3, np.float32).astype(npdt))
    w4f = np.ascontiguousarray(np.asarray(w4, np.float32))
    b3f = np.asarray(b3, np.float32).reshape(1, O)
    id128 = np.eye(128, dtype=np.float32)
    i64rep = np.ascontiguousarray(
        np.tile(np.eye(O, dtype=np.float32), (1, MOCT)).astype(npdt))

    in_maps = []
    for k in range(NCORES):
        bi, q = k // (NCORES // B), k % (NCORES // B)
        in_maps.append({
            "xb": np.ascontiguousarray(x[:, bi, :]),
            "xa": np.ascontiguousarray(x[q * LBLK:(q + 1) * LBLK, bi, :]),
            "w1g": w1g, "w2g": w2g, "b1e": b1e, "b2e": b2e,
            "w3c": w3c, "w4f": w4f, "b3f": b3f,
            "id128": id128, "i64rep": i64rep,
        })
    return in_maps


def kernel_run(inputs, trace=False, mode=None):
    mode = mode or MM_MODE
    nc, npdt = _get_nc(mode)
    in_maps = _make_in_maps(npdt=npdt, **inputs)
    res = run_bass_kernel_spmd(
        nc, in_maps, core_ids=list(range(NCORES)), trace=trace,
    )
    out = np.empty((B, L, L, O), dtype=np.float32)
    for k in range(NCORES):
        bi, q = k // (NCORES // B), k % (NCORES // B)
        out[bi, q * LBLK:(q + 1) * LBLK] = \
            res.results[k]["out"].reshape(LBLK, L, O)
    return out, res


def kernel(**inputs) -> np.ndarray:
    out, _ = kernel_run(inputs, trace=False)
    return out


# revision 9
# speedup vs baseline: 1.2395x; 1.2395x over previous
"""Trainium2 Bass kernel for nn_ComparisonLayer.

Computes, for x:(L,B,D) with L=512,B=2,D=256,C=128,O=64:
    xb  = layernorm(transpose(x,(1,0,2)))          # (B,L,D)
    a   = xb@w1+b1 ; b = xb@w2+b2                  # (B,L,C)
    out[b,l,m,o] = sum_c a[b,l,c]*b[b,m,c]*w3[c,o] + b3[o]
                   + (a@w4)[b,l,o] - (b@w4)[b,m,o] # (B,L,L,O)

Sharding: 8 cores, core k handles batch k//4 and l-block (k%4)*128.
Each core writes out[b, lblk:lblk+128, :, :] = (128, 512*64) with l on
partitions and (m,o) contiguous on the free dim -> contiguous HBM writes.

Per chunk (8 m's = 512 free columns) three matmuls accumulate in PSUM:
  1. main : lhsT=aT (C=128,l=128), rhs=R_chunk[c,(m,o)] = bT[c,m]*w3[c,o]
  2. a4   : lhsT=a4T (O=64,l=128), rhs=I64 tiled (constant)  -> +a4[l,o]
  3. row  : lhsT=ones (1,128),     rhs=(b3-b4) flat slice    -> +b3[o]-b4[m,o]
R_chunk is built on the vector engine with stride-0 broadcast APs.
"""

import os
import numpy as np
import ml_dtypes

import concourse.bacc as bacc
import concourse.bass as bass
import concourse.mybir as mybir
import concourse.tile as tile
from concourse.bass_utils import run_bass_kernel_spmd

L, B, D, C, O = 512, 2, 256, 128, 64
NCORES = 8
LBLK = 128                   # l rows per core
MOCT = 8                     # m values per chunk
CHW = MOCT * O               # 512 = chunk width (free columns)
NCH = L // MOCT              # 64 chunks
LN_EPS = 1e-5

F32 = mybir.dt.float32
BF16 = mybir.dt.bfloat16
F32R = mybir.dt.float32r

# "f32r" (default): matmul operands float32r (full PE rate at N>=512,
# near-fp32 accuracy). "bf16": operands bf16.
MM_MODE = os.environ.get("BASS_MM_MODE", "f32r")


def _build(mode: str):
    # storage dtype of main-matmul operands; producers round on write
    cdt = F32R if mode == "f32r" else BF16
    npdt = np.float32 if mode == "f32r" else ml_dtypes.bfloat16

    nc = bacc.Bacc("TRN2", target_bir_lowering=False, debug=False)

    xb_d = nc.dram_tensor("xb", (L, D), F32, kind="ExternalInput")
    xa_d = nc.dram_tensor("xa", (LBLK, D), F32, kind="ExternalInput")
    w1_d = nc.dram_tensor("w1g", (D, C), F32, kind="ExternalInput")
    w2_d = nc.dram_tensor("w2g", (D, C), F32, kind="ExternalInput")
    b1_d = nc.dram_tensor("b1e", (C, 1), F32, kind="ExternalInput")
    b2_d = nc.dram_tensor("b2e", (C, 1), F32, kind="ExternalInput")
    w3_d = nc.dram_tensor("w3c", (C, O), F32, kind="ExternalInput")
    w4_d = nc.dram_tensor("w4f", (C, O), F32, kind="ExternalInput")
    b3_d = nc.dram_tensor("b3f", (1, O), F32, kind="ExternalInput")
    id128_d = nc.dram_tensor("id128", (128, 128), F32, kind="ExternalInput")
    i64r_d = nc.dram_tensor("i64rep", (O, CHW), cdt, kind="ExternalInput")
    out_d = nc.dram_tensor("out", (LBLK, L * O), F32, kind="ExternalOutput")

    NT = L // 128  # xb tiles

    with tile.TileContext(nc) as tc:
        with (
            tc.tile_pool(name="const", bufs=1) as cp,
            tc.tile_pool(name="work", bufs=2) as wp,
            tc.tile_pool(name="rpool", bufs=6) as rp,
            tc.tile_pool(name="opool", bufs=6) as op,
            tc.tile_pool(name="ps_pre", bufs=4, space="PSUM") as pp,
            tc.tile_pool(name="ps_main", bufs=4, space="PSUM") as pm,
        ):
            # ---------- loads ----------
            xsb = cp.tile([128, NT, D], F32)
            nc.sync.dma_start(xsb[:], xb_d.rearrange("(t p) d -> p t d", p=128))
            xasb = cp.tile([128, D], F32)
            nc.sync.dma_start(xasb[:], xa_d[:])
            w1s = cp.tile([128, 2, C], F32)
            nc.sync.dma_start(w1s[:], w1_d.rearrange("(h p) c -> p h c", p=128))
            w2s = cp.tile([128, 2, C], F32)
            nc.sync.dma_start(w2s[:], w2_d.rearrange("(h p) c -> p h c", p=128))
            b1s = cp.tile([C, 1], F32)
            nc.sync.dma_start(b1s[:], b1_d[:])
            b2s = cp.tile([C, 1], F32)
            nc.sync.dma_start(b2s[:], b2_d[:])
            w3s = cp.tile([C, O], F32)
            nc.sync.dma_start(w3s[:], w3_d[:])
            w4s = cp.tile([C, O], F32)
            nc.sync.dma_start(w4s[:], w4_d[:])
            b3s = cp.tile([1, O], F32)
            nc.sync.dma_start(b3s[:], b3_d[:])
            id128 = cp.tile([128, 128], F32)
            nc.sync.dma_start(id128[:], id128_d[:])
            i64r = cp.tile([O, CHW], cdt)
            nc.sync.dma_start(i64r[:], i64r_d[:])

            epsp = cp.tile([128, 1], F32)
            nc.vector.memset(epsp[:], LN_EPS)
            zerop = cp.tile([128, 1], F32)
            nc.vector.memset(zerop[:], 0.0)
            ones_f = cp.tile([1, 128], F32)
            nc.vector.memset(ones_f[:], 1.0)
            ones_c = cp.tile([1, 128], cdt)
            nc.vector.tensor_copy(ones_c[:], ones_f[:])

            # ---------- layernorm ----------
            def layer_norm(dst, src):
                # dst, src: (128, D) f32
                s = wp.tile([128, 1], F32, tag="ln_s")
                nc.vector.tensor_reduce(
                    s[:], src, axis=mybir.AxisListType.X, op=mybir.AluOpType.add
                )
                nmu = wp.tile([128, 1], F32, tag="ln_nmu")
                nc.scalar.mul(nmu[:], s[:], -1.0 / D)
                cen = wp.tile([128, D], F32, tag="ln_cen")
                nc.scalar.activation(
                    cen[:], src, mybir.ActivationFunctionType.Identity,
                    bias=nmu[:],
                )
                sq = wp.tile([128, D], F32, tag="ln_sq")
                vs = wp.tile([128, 1], F32, tag="ln_vs")
                nc.scalar.activation(
                    sq[:], cen[:], mybir.ActivationFunctionType.Square,
                    bias=zerop[:], accum_out=vs[:],
                )
                std = wp.tile([128, 1], F32, tag="ln_std")
                nc.scalar.activation(
                    std[:], vs[:], mybir.ActivationFunctionType.Sqrt,
                    bias=epsp[:], scale=1.0 / D,
                )
                rstd = wp.tile([128, 1], F32, tag="ln_rstd")
                nc.vector.reciprocal(rstd[:], std[:])
                nc.scalar.activation(
                    dst, cen[:], mybir.ActivationFunctionType.Copy,
                    scale=rstd[:],
                )

            xn = cp.tile([128, NT, D], F32)
            for t in range(NT):
                layer_norm(xn[:, t, :], xsb[:, t, :])
            xna = cp.tile([128, D], F32)
            layer_norm(xna[:], xasb[:])

            # ---------- transposes: xnT (d on partitions) ----------
            xnT = cp.tile([128, 2, L], F32)     # [d_in_half, h, m]
            for t in range(NT):
                for h in range(2):
                    tp = pp.tile([128, 128], F32, tag="pre")
                    nc.tensor.transpose(
                        tp[:], xn[:, t, h * 128:(h + 1) * 128], id128[:]
                    )
                    nc.scalar.copy(xnT[:, h, t * 128:(t + 1) * 128], tp[:])
            xnaT = cp.tile([128, 2, 128], F32)
            for h in range(2):
                tp = pp.tile([128, 128], F32, tag="pre")
                nc.tensor.transpose(
                    tp[:], xna[:, h * 128:(h + 1) * 128], id128[:]
                )
                nc.scalar.copy(xnaT[:, h, :], tp[:])

            # ---------- bT (C, L) and aT (C, 128), fp32 ----------
            bps = pp.tile([C, L], F32, tag="pre")
            for h in range(2):
                nc.tensor.matmul(
                    bps[:], w2s[:, h, :], xnT[:, h, :],
                    start=(h == 0), stop=(h == 1),
                )
            bT = cp.tile([C, L], F32)
            nc.vector.tensor_scalar_add(bT[:], bps[:], b2s[:])

            aps_ = pp.tile([C, 128], F32, tag="pre")
            for h in range(2):
                nc.tensor.matmul(
                    aps_[:], w1s[:, h, :], xnaT[:, h, :],
                    start=(h == 0), stop=(h == 1),
                )
            aT = cp.tile([C, 128], F32)
            nc.vector.tensor_scalar_add(aT[:], aps_[:], b1s[:])

            # main-matmul lhsT in mm dtype (rounded on write)
            aT_c = cp.tile([C, 128], cdt)
            nc.vector.tensor_copy(aT_c[:], aT[:])
            bT_c = bT   # only ever a DVE tensor_tensor input

            # ---------- a4 (l,o) -> a4T (o,l) ----------
            a4ps = pp.tile([128, O], F32, tag="pre")
            nc.tensor.matmul(a4ps[:], aT[:], w4s[:], start=True, stop=True)
            a4sb = cp.tile([128, O], F32)
            nc.scalar.copy(a4sb[:], a4ps[:])
            a4Tps = pp.tile([O, 128], F32, tag="pre")
            nc.tensor.transpose(a4Tps[:], a4sb[:], id128[:])
            a4T = cp.tile([O, 128], cdt)
            nc.scalar.copy(a4T[:], a4Tps[:])

            # ---------- row64 = (b3 - b4) flattened (1, L*O) ----------
            b3cps = pp.tile([128, O], F32, tag="pre")
            nc.tensor.matmul(b3cps[:], ones_f[:], b3s[:], start=True, stop=True)
            b3c = cp.tile([128, O], F32)
            nc.scalar.copy(b3c[:], b3cps[:])

            negb4 = cp.tile([128, NT, O], cdt)
            for mt in range(NT):
                b4ps = pp.tile([128, O], F32, tag="pre")
                nc.tensor.matmul(
                    b4ps[:], bT[:, mt * 128:(mt + 1) * 128], w4s[:],
                    start=True, stop=True,
                )
                nc.vector.tensor_sub(negb4[:, mt, :], b3c[:], b4ps[:])

            row64 = cp.tile([1, L * O], cdt)
            for mt in range(NT):
                dst = row64[0:1, mt * 128 * O:(mt + 1) * 128 * O]
                nc.gpsimd.dma_start(dst, negb4[:, mt, :])

            # ---------- main loop over chunks ----------
            for ch in range(NCH):
                rch = rp.tile([C, MOCT, O], cdt, tag="rch")
                in0 = bT_c[:, ch * MOCT:(ch + 1) * MOCT].unsqueeze(2) \
                    .broadcast_to((C, MOCT, O))
                in1 = w3s[:].unsqueeze(1).broadcast_to((C, MOCT, O))
                nc.vector.tensor_mul(rch[:], in0, in1)

                ps = pm.tile([128, CHW], F32, tag="ps")
                nc.tensor.matmul(ps[:], aT_c[:], rch[:],
                                 start=True, stop=False)
                nc.tensor.matmul(ps[:], a4T[:], i64r[:],
                                 start=False, stop=False)
                nc.tensor.matmul(
                    ps[:], ones_c[:],
                    row64[0:1, ch * CHW:(ch + 1) * CHW],
                    start=False, stop=True,
                )

                ob = op.tile([128, CHW], F32, tag="ob")
                nc.scalar.copy(ob[:], ps[:])
                nc.sync.dma_start(out_d[:, ch * CHW:(ch + 1) * CHW], ob[:])

    nc.compile()
    return nc, npdt


_CACHE = {}


def _get_nc(mode):
    if mode not in _CACHE:
        _CACHE[mode] = _build(mode)
    return _CACHE[mode]


def _make_in_maps(x, ln_gamma, ln_beta, w1, b1, w2, b2, w3, b3, w4, npdt):
    x = np.ascontiguousarray(x, dtype=np.float32)
    g = np.asarray(ln_gamma, np.float32)
    be = np.asarray(ln_beta, np.float32)
    w1 = np.asarray(w1, np.float32)
    w2 = np.asarray(w2, np.float32)
    # fold the LN affine into the first-layer weights:
    # (xn*g + be) @ w = xn @ (g[:,None]*w) + be @ w
    w1g = g[:, None] * w1
    w2g = g[:, None] * w2
    b1e = (np.asarray(b1, np.float32) + be @ w1).reshape(C, 1)
    b2e = (np.asarray(b2, np.float32) + be @ w2).reshape(C, 1)
    w3c = np.ascontiguousarray(np.asarray(w3, np.float32))
    w4f = np.ascontiguousarray(np.asarray(w4, np.float32))
    b3f = np.asarray(b3, np.float32).reshape(1, O)
    id128 = np.eye(128, dtype=np.float32)
    i64rep = np.ascontiguousarray(
        np.tile(np.eye(O, dtype=np.float32), (1, MOCT)).astype(npdt))

    in_maps = []
    for k in range(NCORES):
        bi, q = k // (NCORES // B), k % (NCORES // B)
        in_maps.append({
            "xb": np.ascontiguousarray(x[:, bi, :]),
            "xa": np.ascontiguousarray(x[q * LBLK:(q + 1) * LBLK, bi, :]),
            "w1g": w1g, "w2g": w2g, "b1e": b1e, "b2e": b2e,
            "w3c": w3c, "w4f": w4f, "b3f": b3f,
            "id128": id128, "i64rep": i64rep,
        })
    return in_maps


def kernel_run(inputs, trace=False, mode=None):
    mode = mode or MM_MODE
    nc, npdt = _get_nc(mode)
    in_maps = _make_in_maps(npdt=npdt, **inputs)
    res = run_bass_kernel_spmd(
        nc, in_maps, core_ids=list(range(NCORES)), trace=trace,
    )
    out = np.empty((B, L, L, O), dtype=np.float32)
    for k in range(NCORES):
        bi, q = k // (NCORES // B), k % (NCORES // B)
        out[bi, q * LBLK:(q + 1) * LBLK] = \
            res.results[k]["out"].reshape(LBLK, L, O)
    return out, res


def kernel(**inputs) -> np.ndarray:
    out, _ = kernel_run(inputs, trace=False)
    return out
